# revision 16
# baseline (speedup 1.0000x reference)
"""Trainium2 Bass kernel: 3D interpolation (2x bilinear in H,W + 2x nearest in D).

Input  x: (2, 1, 128, 128, 128) f32
Output  : (2, 1, 256, 256, 256) f32

Math (scale=2, align_corners=False): separable 2-tap filter {0.75, 0.25}:
  row 2p   = 0.25*x[p-1] + 0.75*x[p]   (clamped at p=0)
  row 2p+1 = 0.75*x[p]   + 0.25*x[p+1] (clamped at p=H-1)
applied along H then W; the D axis is a pure repeat (each plane written twice).

Sharding: pure data-parallel over the 256 (b, d) slices -> 32 slices/core on
8 cores; no communication.

Key hardware facts this design is built around (measured on-device):
  - HWDGE DMAs fan out across the 16 SDMA engines ONLY for 128-partition
    SBUF-side access patterns; any sub-128-partition DMA becomes a
    single-engine descriptor chain (~28 GB/s). So every bulk DMA here is
    exactly 128 partitions.
  - Compute-engine APs must start at partition offsets that are multiples
    of 32, so the +-1 partition shifts for the H filter cannot be done with
    shifted operands. They are done on the idle TensorEngine instead:
    xup = S_up.T @ x, xdn = S_dn.T @ x with 0/1 shift matrices (exact in
    fp32, clamp rows baked in), landing in otherwise-unused PSUM.
  - Output rows are paired (2p, 2p+1) per partition in one merged tile M so
    each DMA descriptor covers a contiguous 2 KiB DRAM run.

Per-core pipeline (S=8 slices per iteration, 4 iterations):
  load x -> PE shift matmuls -> H-stage (ACT scale + DVE stt) -> W-stage
  (free-axis shifted stt, stride-2 interleaved writes into M) -> 2 store
  DMAs (D-repeat).
"""
import numpy as np

N_CORES = 8
B, D, H, W = 2, 128, 128, 128
SLICES_PER_CORE = (B * D) // N_CORES  # 32
ITER_SIZES = (2, 6, 8, 8, 6, 2)       # slices per pipeline iteration
assert sum(ITER_SIZES) == SLICES_PER_CORE

_cache = {}


def _shift_weights():
    """(128, 256) f32: [:, 0:128] = S_up, [:, 128:256] = S_dn (as lhsT).

    matmul(out, lhsT, rhs) = lhsT.T @ rhs, so out[m] = sum_k lhsT[k, m] x[k].
    S_up: out[m] = x[m+1] (m<=126), out[127] = x[127].
    S_dn: out[m] = x[m-1] (m>=1),  out[0]   = x[0].
    """
    w = np.zeros((H, 2 * H), np.float32)
    k = np.arange(1, H)
    w[k, k - 1] = 1.0
    w[H - 1, H - 1] = 1.0
    k = np.arange(0, H - 1)
    w[k, H + k + 1] = 1.0
    w[0, H] = 1.0
    return w


def _build():
    from concourse import bacc, mybir
    from concourse.tile import TileContext

    F32 = mybir.dt.float32
    Copy = mybir.ActivationFunctionType.Copy
    mult, add = mybir.AluOpType.mult, mybir.AluOpType.add

    nc = bacc.Bacc("TRN2", target_bir_lowering=False, debug=False)
    x_ext = nc.declare_dram_parameter(
        "x", [SLICES_PER_CORE, H, W], F32, isOutput=False)
    w_ext = nc.declare_dram_parameter("w", [H, 2 * H], F32, isOutput=False)
    y_ext = nc.declare_dram_parameter(
        "y", [2 * SLICES_PER_CORE, 2 * H, 2 * W], F32, isOutput=True)

    def stt(out, in0, s, in1):
        nc.vector.scalar_tensor_tensor(
            out=out, in0=in0, scalar=s, in1=in1, op0=mult, op1=add)

    with TileContext(nc) as tc:
        with tc.tile_pool(name="wpool", bufs=1) as wpool, \
             tc.tile_pool(name="pool", bufs=4) as pool, \
             tc.tile_pool(name="ppool", bufs=2, space="PSUM") as ppool:
            wt = wpool.tile([H, 2 * H], F32)
            nc.sync.dma_start(out=wt[:], in_=w_ext[:])

            start = 0
            for i, S in enumerate(ITER_SIZES):
                sl = slice(start, start + S)
                xt = pool.tile([H, S, W], F32, tag="xt")
                xup = ppool.tile([H, S, W], F32, tag="xup")
                xdn = ppool.tile([H, S, W], F32, tag="xdn")
                t3 = pool.tile([H, S, W], F32, tag="t3")

                E = pool.tile([H, S, W], F32, tag="E")
                O = pool.tile([H, S, W], F32, tag="O")
                u3e = pool.tile([H, S, W], F32, tag="u3e")
                u3o = pool.tile([H, S, W], F32, tag="u3o")
                M = pool.tile([H, S, 4 * W], F32, tag="M")

                # load: DRAM (s, h, w) iterated as (h, s, w) to match SBUF
                nc.scalar.dma_start(
                    out=xt[:], in_=x_ext[sl].rearrange("s p w -> p s w"))

                # partition shifts on the TensorEngine (N<=512 fp32 chunks)
                for ps, coff in ((xup, 0), (xdn, H)):
                    for c in range((S + 3) // 4):
                        cs = slice(c * 4, min(c * 4 + 4, S))
                        nc.tensor.matmul(
                            ps[:, cs, :], wt[:, coff:coff + H], xt[:, cs, :],
                            start=True, stop=True)

                # H-stage: E[p] = row 2p, O[p] = row 2p+1
                nc.scalar.activation(t3[:], xt[:], Copy, scale=0.75)
                stt(E[:], xdn[:], 0.25, t3[:])
                stt(O[:], xup[:], 0.25, t3[:])

                # W-stage into merged M: cols 0:2W = even row 2p (E),
                # cols 2W:4W = odd row 2p+1 (O)
                nc.scalar.activation(u3e[:], E[:], Copy, scale=0.75)
                nc.scalar.activation(u3o[:], O[:], Copy, scale=0.75)
                for T, u3, off in ((E, u3e, 0), (O, u3o, 2 * W)):
                    # odd cols 2j+1 (j=0..W-2): 0.75*T[j] + 0.25*T[j+1]
                    stt(M[:, :, off + 1:off + 2 * W - 1:2],
                        T[:, :, 1:W], 0.25, u3[:, :, 0:W - 1])
                    # even cols 2j (j=1..W-1): 0.25*T[j-1] + 0.75*T[j]
                    stt(M[:, :, off + 2:off + 2 * W:2],
                        T[:, :, 0:W - 1], 0.25, u3[:, :, 1:W])
                    nc.scalar.activation(
                        M[:, :, off:off + 1], T[:, :, 0:1], Copy)
                    nc.scalar.activation(
                        M[:, :, off + 2 * W - 1:off + 2 * W],
                        T[:, :, W - 1:W], Copy)

                # stores (x2 for the D-repeat): row pairs (2p, 2p+1)
                for r in range(2):
                    base = 2 * start + r
                    eng = nc.sync if r == 0 else nc.scalar
                    eng.dma_start(
                        out=y_ext[base:base + 2 * S - 1:2]
                        .rearrange("s (p t) w -> p s (t w)", p=H),
                        in_=M[:])
                start += S

    nc.finalize()
    return nc


def _get_nc():
    if "nc" not in _cache:
        _cache["nc"] = _build()
    return _cache["nc"]


def _run(x, trace=False, **kw):
    from concourse.bass_utils import run_bass_kernel_spmd

    nc = _get_nc()
    x = np.asarray(x, dtype=np.float32)
    xr = x.reshape(B * D, H, W)
    w = _shift_weights()
    in_maps = [
        {"x": np.ascontiguousarray(
            xr[k * SLICES_PER_CORE:(k + 1) * SLICES_PER_CORE]),
         "w": w}
        for k in range(N_CORES)
    ]
    bkr = run_bass_kernel_spmd(nc, in_maps, list(range(N_CORES)),
                               trace=trace, **kw)
    out = np.empty((B, 2 * D, 2 * H, 2 * W), dtype=np.float32)
    for k in range(N_CORES):
        g = k * SLICES_PER_CORE
        b, d0 = g // D, g % D
        out[b, 2 * d0:2 * d0 + 2 * SLICES_PER_CORE] = bkr.results[k]["y"]
    return out.reshape(B, 1, 2 * D, 2 * H, 2 * W), bkr


def kernel(x):
    return _run(x)[0]


# revision 17
# speedup vs baseline: 1.0943x; 1.0943x over previous
"""Trainium2 Bass kernel: 3D interpolation (2x bilinear in H,W + 2x nearest in D).

Input  x: (2, 1, 128, 128, 128) f32
Output  : (2, 1, 256, 256, 256) f32

Math (scale=2, align_corners=False): separable 2-tap filter {0.75, 0.25}:
  row 2p   = 0.25*x[p-1] + 0.75*x[p]   (clamped at p=0)
  row 2p+1 = 0.75*x[p]   + 0.25*x[p+1] (clamped at p=H-1)
applied along H then W; the D axis is a pure repeat (each plane written twice).

Sharding: pure data-parallel over the 256 (b, d) slices -> 32 slices/core on
8 cores; no communication.

Key hardware facts this design is built around (measured on-device):
  - HWDGE DMAs fan out across the 16 SDMA engines ONLY for 128-partition
    SBUF-side access patterns; any sub-128-partition DMA becomes a
    single-engine descriptor chain (~28 GB/s). So every bulk DMA here is
    exactly 128 partitions.
  - Compute-engine APs must start at partition offsets that are multiples
    of 32, so the +-1 partition shifts for the H filter cannot be done with
    shifted operands. They are done on the idle TensorEngine instead:
    xup = S_up.T @ x, xdn = S_dn.T @ x with 0/1 shift matrices (exact in
    fp32, clamp rows baked in), landing in otherwise-unused PSUM.
  - Output rows are paired (2p, 2p+1) per partition in one merged tile M so
    each DMA descriptor covers a contiguous 2 KiB DRAM run.

Per-core pipeline (S=8 slices per iteration, 4 iterations):
  load x -> PE shift matmuls -> H-stage (ACT scale + DVE stt) -> W-stage
  (free-axis shifted stt, stride-2 interleaved writes into M) -> 2 store
  DMAs (D-repeat).
"""
import numpy as np

N_CORES = 8
B, D, H, W = 2, 128, 128, 128
SLICES_PER_CORE = (B * D) // N_CORES  # 32
ITER_SIZES = (1, 3, 6, 8, 8, 4, 2)    # slices per pipeline iteration
assert sum(ITER_SIZES) == SLICES_PER_CORE

_cache = {}


def _shift_weights():
    """(128, 256) f32: [:, 0:128] = S_up, [:, 128:256] = S_dn (as lhsT).

    matmul(out, lhsT, rhs) = lhsT.T @ rhs, so out[m] = sum_k lhsT[k, m] x[k].
    S_up: out[m] = x[m+1] (m<=126), out[127] = x[127].
    S_dn: out[m] = x[m-1] (m>=1),  out[0]   = x[0].
    """
    w = np.zeros((H, 2 * H), np.float32)
    k = np.arange(1, H)
    w[k, k - 1] = 1.0
    w[H - 1, H - 1] = 1.0
    k = np.arange(0, H - 1)
    w[k, H + k + 1] = 1.0
    w[0, H] = 1.0
    return w


def _build():
    from concourse import bacc, mybir
    from concourse.tile import TileContext

    F32 = mybir.dt.float32
    Copy = mybir.ActivationFunctionType.Copy
    mult, add = mybir.AluOpType.mult, mybir.AluOpType.add

    nc = bacc.Bacc("TRN2", target_bir_lowering=False, debug=False)
    x_ext = nc.declare_dram_parameter(
        "x", [SLICES_PER_CORE, H, W], F32, isOutput=False)
    w_ext = nc.declare_dram_parameter("w", [H, 2 * H], F32, isOutput=False)
    y_ext = nc.declare_dram_parameter(
        "y", [2 * SLICES_PER_CORE, 2 * H, 2 * W], F32, isOutput=True)

    def stt(out, in0, s, in1):
        nc.vector.scalar_tensor_tensor(
            out=out, in0=in0, scalar=s, in1=in1, op0=mult, op1=add)

    with TileContext(nc) as tc:
        with tc.tile_pool(name="wpool", bufs=1) as wpool, \
             tc.tile_pool(name="pool", bufs=4) as pool, \
             tc.tile_pool(name="ppool", bufs=2, space="PSUM") as ppool:
            wt = wpool.tile([H, 2 * H], F32)
            nc.sync.dma_start(out=wt[:], in_=w_ext[:])

            start = 0
            for i, S in enumerate(ITER_SIZES):
                sl = slice(start, start + S)
                xt = pool.tile([H, S, W], F32, tag="xt")
                xup = ppool.tile([H, S, W], F32, tag="xup")
                xdn = ppool.tile([H, S, W], F32, tag="xdn")
                t3 = pool.tile([H, S, W], F32, tag="t3")

                E = pool.tile([H, S, W], F32, tag="E")
                O = pool.tile([H, S, W], F32, tag="O")
                u3e = pool.tile([H, S, W], F32, tag="u3e")
                u3o = pool.tile([H, S, W], F32, tag="u3o")
                M = pool.tile([H, S, 4 * W], F32, tag="M")

                # load: DRAM (s, h, w) iterated as (h, s, w) to match SBUF
                nc.scalar.dma_start(
                    out=xt[:], in_=x_ext[sl].rearrange("s p w -> p s w"))

                # partition shifts on the TensorEngine (N<=512 fp32 chunks)
                for ps, coff in ((xup, 0), (xdn, H)):
                    for c in range((S + 3) // 4):
                        cs = slice(c * 4, min(c * 4 + 4, S))
                        nc.tensor.matmul(
                            ps[:, cs, :], wt[:, coff:coff + H], xt[:, cs, :],
                            start=True, stop=True)

                # H-stage: E[p] = row 2p, O[p] = row 2p+1
                nc.scalar.activation(t3[:], xt[:], Copy, scale=0.75)
                stt(E[:], xdn[:], 0.25, t3[:])
                stt(O[:], xup[:], 0.25, t3[:])

                # W-stage into merged M: cols 0:2W = even row 2p (E),
                # cols 2W:4W = odd row 2p+1 (O)
                nc.scalar.activation(u3e[:], E[:], Copy, scale=0.75)
                nc.scalar.activation(u3o[:], O[:], Copy, scale=0.75)
                for T, u3, off in ((E, u3e, 0), (O, u3o, 2 * W)):
                    # odd cols 2j+1 (j=0..W-2): 0.75*T[j] + 0.25*T[j+1]
                    stt(M[:, :, off + 1:off + 2 * W - 1:2],
                        T[:, :, 1:W], 0.25, u3[:, :, 0:W - 1])
                    # even cols 2j (j=1..W-1): 0.25*T[j-1] + 0.75*T[j]
                    stt(M[:, :, off + 2:off + 2 * W:2],
                        T[:, :, 0:W - 1], 0.25, u3[:, :, 1:W])
                    nc.scalar.activation(
                        M[:, :, off:off + 1], T[:, :, 0:1], Copy)
                    nc.scalar.activation(
                        M[:, :, off + 2 * W - 1:off + 2 * W],
                        T[:, :, W - 1:W], Copy)

                # stores (x2 for the D-repeat): row pairs (2p, 2p+1)
                for r in range(2):
                    base = 2 * start + r
                    nc.sync.dma_start(
                        out=y_ext[base:base + 2 * S - 1:2]
                        .rearrange("s (p t) w -> p s (t w)", p=H),
                        in_=M[:])
                start += S

    nc.finalize()
    return nc


def _get_nc():
    if "nc" not in _cache:
        _cache["nc"] = _build()
    return _cache["nc"]


def _run(x, trace=False, **kw):
    from concourse.bass_utils import run_bass_kernel_spmd

    nc = _get_nc()
    x = np.asarray(x, dtype=np.float32)
    xr = x.reshape(B * D, H, W)
    w = _shift_weights()
    in_maps = [
        {"x": np.ascontiguousarray(
            xr[k * SLICES_PER_CORE:(k + 1) * SLICES_PER_CORE]),
         "w": w}
        for k in range(N_CORES)
    ]
    bkr = run_bass_kernel_spmd(nc, in_maps, list(range(N_CORES)),
                               trace=trace, **kw)
    out = np.empty((B, 2 * D, 2 * H, 2 * W), dtype=np.float32)
    for k in range(N_CORES):
        g = k * SLICES_PER_CORE
        b, d0 = g // D, g % D
        out[b, 2 * d0:2 * d0 + 2 * SLICES_PER_CORE] = bkr.results[k]["y"]
    return out.reshape(B, 1, 2 * D, 2 * H, 2 * W), bkr


def kernel(x):
    return _run(x)[0]
